# revision 2
# baseline (speedup 1.0000x reference)
"""Multi-head attention (B=4, S=2048, DM=1024, H=16, DH=64) on 8 TRN2 cores.

Sharding: 8 cores = 4 batches x 2 head-halves. Core c handles batch c//2 and
heads [ (c%2)*8, (c%2)*8+8 ).  Each core projects Q/K/V for its 8 heads,
runs causal softmax attention, applies its slice of w_o, and writes a partial
[S, DM] output.  The host sums the two partials per batch.

Attention layout (v2): logits are computed transposed ([kv, q]) per head pair,
exponentiated into an SBUF buffer, and the PV product runs in the *flipped*
orientation -- exp tile as the stationary operand, V (with a fused ones column
for the softmax denominator) as the moving operand -- so each PV matmul
streams only 65 columns instead of 512.  The resulting [q, head-dim] tiles
are normalized with per-partition reciprocal scalars, transposed back to
[head-dim, q] on the tensor engine, and fed to the w_o projection.

All matmuls run in bf16 with fp32 PSUM accumulation; logits skip row-max
subtraction (inputs are O(1) so exp cannot overflow).
"""

import math

import ml_dtypes
import numpy as np

B, S, DM, H, DH = 4, 2048, 1024, 16, 64
NCORES = 8
HPC = H // 2        # heads per core
PAIRS = HPC // 2    # head pairs per core (packed 2-per-128-partitions)
F = 512             # query block (free dim of QK matmuls)
CH = 128            # kv chunk (partition dim of transposed logits)
NQB = S // F        # query blocks
NT = S // CH        # kv chunks
VE = DH + 1         # V extended with a ones column (fused denominator)
KT = DM // 128      # contraction k-tiles for projections
KO = HPC * DH // 128  # contraction k-tiles for w_o
NST = S // CH       # query sub-tiles of 128 (same granularity as kv chunks)
SCALE = 1.0 / math.sqrt(DH)

_CACHE = {}


def _split_excess_waits(nc):
    """This environment's walrus rejects instructions carrying more than one
    sync wait ("Too many sync wait commands").  Hoist excess waits onto
    single-wait NoOps inserted right before the offending instruction."""
    import concourse.mybir as mybir

    n = 0
    for f in nc.m.functions:
        for blk in f.blocks:
            newlist = []
            for ins in blk.instructions:
                si = ins.sync_info
                if si is not None and len(si.on_wait) > 1:
                    for w in si.on_wait[:-1]:
                        n += 1
                        newlist.append(
                            mybir.InstNoOp(
                                name=f"I-waitfix-{n}",
                                opcode="NoOp",
                                engine=ins.engine,
                                sync_info=mybir.SyncInfo(on_wait=[w], on_update=[]),
                            )
                        )
                    si.on_wait = si.on_wait[-1:]
                newlist.append(ins)
            blk.instructions = newlist
    return n


def _build(causal, reps=1):
    import concourse.bass as bass
    import concourse.mybir as mybir
    import concourse.tile as tile

    bf16 = mybir.dt.bfloat16
    f32 = mybir.dt.float32
    Exp = mybir.ActivationFunctionType.Exp

    nc = bass.Bass()
    et = nc.dram_tensor("et", [DM, S], bf16, kind="ExternalInput")
    wq = nc.dram_tensor("wq", [DM, HPC * DH], bf16, kind="ExternalInput")
    wk = nc.dram_tensor("wk", [DM, HPC * DH], bf16, kind="ExternalInput")
    wv = nc.dram_tensor("wv", [DM, HPC * DH], bf16, kind="ExternalInput")
    wo = nc.dram_tensor("wo", [HPC * DH, DM], bf16, kind="ExternalInput")
    band = nc.dram_tensor("band", [CH, 2 * F], bf16, kind="ExternalInput")
    ident = nc.dram_tensor("ident", [128, 128], f32, kind="ExternalInput")
    out = nc.dram_tensor("out", [S, DM], f32, kind="ExternalOutput")

    with tile.TileContext(nc) as tc:
        with tc.tile_pool(name="const", bufs=1) as cpool, \
             tc.tile_pool(name="qk", bufs=2) as qkpool, \
             tc.tile_pool(name="eexp", bufs=2) as epool, \
             tc.tile_pool(name="hn", bufs=3) as hnpool, \
             tc.tile_pool(name="outp", bufs=2) as opool, \
             tc.tile_pool(name="small", bufs=3) as spool, \
             tc.tile_pool(name="ps", bufs=1, space="PSUM") as ps:

            # ---- constant loads (few large DMAs; order matters for startup)
            wv_t = cpool.tile([128, KT * HPC * DH], bf16, name="wv_t")
            nc.sync.dma_start(
                wv_t.rearrange("p (a n) -> p a n", a=KT),
                wv.rearrange("(a p) n -> p a n", p=128),
            )
            et_t = cpool.tile([128, KT * S], bf16, name="et_t")
            NQ4 = S // 4
            et_t3 = et_t.rearrange("p (a s) -> p a s", a=KT)
            et3 = et.rearrange("(a p) s -> p a s", p=128)
            nc.sync.dma_start(
                et_t3[:, :, 0 * NQ4 : 1 * NQ4], et3[:, :, 0 * NQ4 : 1 * NQ4]
            )
            w_tiles = {}
            for nm, src in (("wk", wk), ("wq", wq)):
                t = cpool.tile([128, KT * HPC * DH], bf16, name=f"{nm}_t")
                nc.sync.dma_start(
                    t.rearrange("p (a n) -> p a n", a=KT),
                    src.rearrange("(a p) n -> p a n", p=128),
                )
                w_tiles[nm] = t
            wq_t, wk_t = w_tiles["wq"], w_tiles["wk"]
            for cq in range(1, 4):
                nc.sync.dma_start(
                    et_t3[:, :, cq * NQ4 : (cq + 1) * NQ4],
                    et3[:, :, cq * NQ4 : (cq + 1) * NQ4],
                )
            wo_t = cpool.tile([128, KO * DM], bf16, name="wo_t")
            nc.sync.dma_start(
                wo_t.rearrange("p (a n) -> p a n", a=KO),
                wo.rearrange("(a p) n -> p a n", p=128),
            )
            band_t = cpool.tile([CH, 2 * F], bf16, name="band_t")
            nc.sync.dma_start(band_t[:], band[:])
            ident_t = cpool.tile([128, 128], f32, name="ident_t")
            nc.sync.dma_start(ident_t[:], ident[:])

            for _rep in range(reps):
                # V projection target: per kv chunk i, 8 heads x (64 + ones)
                vsb = cpool.tile([128, NT * HPC * VE], bf16, name="vsb")
                nc.vector.memset(
                    vsb.rearrange("p (i e) -> p i e", e=VE)[:, :, DH:VE], 1.0
                )
                headsT = [
                    cpool.tile([128, S], bf16, name=f"headsT{t}", tag=f"headsT{t}")
                    for t in range(PAIRS)
                ]

                def vblock(i):
                    """V projection for kv chunk i (all 8 heads)."""
                    vps = ps.tile([128, 512], f32, tag="mm512", bufs=2, name="vps")
                    for kt in range(KT):
                        nc.tensor.matmul(
                            vps[:],
                            et_t[:, kt * S + i * CH : kt * S + (i + 1) * CH],
                            wv_t[:, kt * HPC * DH : (kt + 1) * HPC * DH],
                            start=(kt == 0),
                            stop=(kt == KT - 1),
                        )
                    nc.vector.tensor_copy(
                        vsb[:, i * HPC * VE : (i + 1) * HPC * VE].rearrange(
                            "p (h e) -> p h e", e=VE
                        )[:, :, 0:DH],
                        vps.rearrange("p (h d) -> p h d", d=DH),
                    )

                qk_tiles = {}

                def qkblock(p, which, j):
                    """Q or K projection for pair p, query block j -> [hd, q]."""
                    key = (p, which)
                    if key not in qk_tiles:
                        qk_tiles[key] = qkpool.tile(
                            [128, S], bf16, tag=f"{which}t2", name=f"{which}t2"
                        )
                    dst = qk_tiles[key]
                    wt = wq_t if which == "q" else wk_t
                    pps = ps.tile([128, 512], f32, tag="mm512", bufs=2, name="pps")
                    for kt in range(KT):
                        nc.tensor.matmul(
                            pps[:],
                            wt[:, kt * HPC * DH + p * 128 : kt * HPC * DH + (p + 1) * 128],
                            et_t[:, kt * S + j * F : kt * S + (j + 1) * F],
                            start=(kt == 0),
                            stop=(kt == KT - 1),
                        )
                    nc.vector.tensor_copy(dst[:, j * F : (j + 1) * F], pps[:])
                    return dst

                def wo_block(st):
                    """Output projection for query sub-tile st (128 queries)."""
                    ot = opool.tile([128, DM], f32, tag="ot", name="ot")
                    for nh in range(2):
                        wps = ps.tile([128, 512], f32, tag="mm512", bufs=2, name="wps")
                        for ktt in range(KO):
                            nc.tensor.matmul(
                                wps[:],
                                headsT[ktt][:, st * CH : (st + 1) * CH],
                                wo_t[:, ktt * DM + nh * 512 : ktt * DM + (nh + 1) * 512],
                                start=(ktt == 0),
                                stop=(ktt == KO - 1),
                            )
                        nc.vector.tensor_copy(ot[:, nh * 512 : (nh + 1) * 512], wps[:])
                    nc.sync.dma_start(out[st * CH : (st + 1) * CH, :], ot[:])

                # startup: just enough projections for pair 0 / PV of qb 0
                qkblock(0, "k", 0)
                qkblock(0, "q", 0)
                for i in range(4):
                    vblock(i)

                for p in range(PAIRS):
                    qt2 = qk_tiles[(p, "q")]
                    kt2 = qk_tiles[(p, "k")]

                    for qb in range(NQB):
                        nch = 4 * qb + 4 if causal else NT

                        # r0: first causally-live query column within this
                        # block for chunk c (narrowed band computation)
                        def _r0(c):
                            return (c - 4 * qb) * CH if causal and c >= 4 * qb else 0

                        e_grp = epool.tile(
                            [128, NT * 2 * F], bf16, tag="e", name="e_grp"
                        )
                        # ---- QK + exp per kv chunk
                        for c in range(nch):
                            r0 = _r0(c)
                            stg = ps.tile([128, 2 * F], f32, tag="stg", bufs=2, name="stg")
                            for hh in (0, 1):
                                nc.tensor.matmul(
                                    stg[:, hh * F + r0 : (hh + 1) * F],
                                    kt2[64 * hh : 64 * hh + 64, c * CH : (c + 1) * CH],
                                    qt2[64 * hh : 64 * hh + 64, qb * F + r0 : (qb + 1) * F],
                                    start=True,
                                    stop=True,
                                )
                            st3 = stg.rearrange("p (h f) -> p h f", h=2)[:, :, r0:F]
                            ex3 = e_grp[:, 2 * c * F : (2 * c + 2) * F].rearrange(
                                "p (h f) -> p h f", h=2
                            )[:, :, r0:F]
                            nc.scalar.activation(ex3, st3, Exp, scale=SCALE)
                            if causal and c >= 4 * qb:
                                for hh in (0, 1):
                                    sl = e_grp[:, (2 * c + hh) * F + r0 : (2 * c + hh + 1) * F]
                                    nc.vector.tensor_mul(
                                        sl, sl, band_t[:, F : 2 * F - r0]
                                    )

                        # ---- flipped PV per query sub-tile of 128, then
                        # normalize, transpose back, and (last pair) w_o
                        for stl in range(F // CH):
                            st = qb * (F // CH) + stl
                            ncv = 4 * qb + stl + 1 if causal else NT
                            pv = ps.tile([128, 2 * VE], f32, tag="pv", bufs=2, name="pv")
                            for c in range(ncv):
                                for hh in (0, 1):
                                    nc.tensor.matmul(
                                        pv[:, hh * VE : (hh + 1) * VE],
                                        e_grp[:, (2 * c + hh) * F + stl * CH : (2 * c + hh) * F + (stl + 1) * CH],
                                        vsb[:, c * HPC * VE + (2 * p + hh) * VE : c * HPC * VE + (2 * p + hh + 1) * VE],
                                        start=(c == 0),
                                        stop=(c == ncv - 1),
                                    )
                            pv3 = pv.rearrange("p (h e) -> p h e", e=VE)
                            rec = spool.tile([128, 2], f32, tag="rec", name="rec")
                            hn = hnpool.tile([128, 128], f32, tag="hn", name="hn")
                            nc.vector.reciprocal(rec[:], pv3[:, :, DH : DH + 1])
                            for hh in (0, 1):
                                nc.vector.tensor_scalar_mul(
                                    hn[:, hh * DH : (hh + 1) * DH],
                                    pv3[:, hh, 0:DH],
                                    rec[:, hh : hh + 1],
                                )
                            tp = ps.tile([128, 512], f32, tag="mm512", bufs=2, name="tp")
                            nc.tensor.transpose(tp[:, 0:128], hn[:], ident_t[:])
                            nc.vector.tensor_copy(
                                headsT[p][:, st * CH : (st + 1) * CH], tp[:, 0:128]
                            )
                            if p == PAIRS - 1:
                                wo_block(st)

                        # ---- fillers: projections for the next pair / V
                        if p == 0 and causal and qb < NQB - 1:
                            for i in range(4 * qb + 4, 4 * qb + 8):
                                vblock(i)
                        if p < PAIRS - 1:
                            qkblock(p + 1, "k", qb)
                            qkblock(p + 1, "q", qb)
                    if p == 0 and causal:
                        for i in range(12, 16):
                            vblock(i)
                    if not causal and p == 0:
                        for i in range(4, NT):
                            vblock(i)

    _split_excess_waits(nc)
    return nc


def _get_nc(causal):
    key = ("nc", causal)
    if key not in _CACHE:
        _CACHE[key] = _build(causal)
    return _CACHE[key]


def _host_inputs(embed, w_q, w_k, w_v, w_o):
    """Per-core input dicts (bf16 pre-cast / pre-transposed on host)."""
    bf = ml_dtypes.bfloat16
    band = (np.arange(CH)[:, None] <= np.arange(2 * F)[None, :] - F).astype(bf)
    ident = np.eye(128, dtype=np.float32)
    ins = []
    for c in range(NCORES):
        b, half = divmod(c, 2)
        h0 = half * HPC
        ins.append(
            {
                "et": np.ascontiguousarray(embed[b].T).astype(bf),
                "wq": np.ascontiguousarray(
                    w_q[h0 : h0 + HPC].transpose(1, 0, 2).reshape(DM, HPC * DH)
                ).astype(bf),
                "wk": np.ascontiguousarray(
                    w_k[h0 : h0 + HPC].transpose(1, 0, 2).reshape(DM, HPC * DH)
                ).astype(bf),
                "wv": np.ascontiguousarray(
                    w_v[h0 : h0 + HPC].transpose(1, 0, 2).reshape(DM, HPC * DH)
                ).astype(bf),
                "wo": np.ascontiguousarray(w_o[h0 * DH : (h0 + HPC) * DH]).astype(bf),
                "band": band,
                "ident": ident,
            }
        )
    return ins


def _numpy_fallback(embed, mask, w_q, w_k, w_v, w_o):
    """Exact fp32 host computation for mask patterns the device kernel does
    not implement (never hit for the reference's causal mask)."""
    out = np.zeros((B, S, DM), np.float32)
    for b in range(B):
        heads = np.zeros((S, H * DH), np.float32)
        for h in range(H):
            q = embed[b] @ w_q[h]
            k = embed[b] @ w_k[h]
            v = embed[b] @ w_v[h]
            logits = (q @ k.T) * SCALE
            logits = np.where(mask[b], logits, -np.inf)
            logits -= logits.max(axis=-1, keepdims=True)
            p = np.exp(logits)
            p /= p.sum(axis=-1, keepdims=True)
            heads[:, h * DH : (h + 1) * DH] = p @ v
        out[b] = heads @ w_o
    return out


def _get_runner(causal):
    """Cached jitted sharded executor for the built Bass module.

    Mirrors bass2jax.run_bass_via_pjrt's multi-core path, but keeps the
    jitted callable so repeated kernel() calls skip re-tracing/compiling."""
    key = ("runner", causal)
    if key in _CACHE:
        return _CACHE[key]

    import jax
    from jax.experimental.shard_map import shard_map
    from jax.sharding import Mesh, PartitionSpec

    import concourse.mybir as mybir
    from concourse import bass2jax

    bass2jax.install_neuronx_cc_hook()
    nc = _get_nc(causal)
    partition_name = nc.partition_id_tensor.name if nc.partition_id_tensor else None
    in_names, out_names, out_avals, out_shapes = [], [], [], []
    for alloc in nc.m.functions[0].allocations:
        if not isinstance(alloc, mybir.MemoryLocationSet):
            continue
        name = alloc.memorylocations[0].name
        if alloc.kind == "ExternalInput":
            if name != partition_name:
                in_names.append(name)
        elif alloc.kind == "ExternalOutput":
            shape = tuple(alloc.tensor_shape)
            dtype = mybir.dt.np(alloc.dtype)
            out_names.append(name)
            out_avals.append(jax.core.ShapedArray(shape, dtype))
            out_shapes.append((shape, dtype))
    n_params = len(in_names)
    all_in_names = list(in_names) + list(out_names)
    if partition_name is not None:
        all_in_names.append(partition_name)

    def _body(*args):
        operands = list(args)
        if partition_name is not None:
            operands.append(bass2jax.partition_id_tensor())
        return tuple(
            bass2jax._bass_exec_p.bind(
                *operands,
                out_avals=tuple(out_avals),
                in_names=tuple(all_in_names),
                out_names=tuple(out_names),
                lowering_input_output_aliases=(),
                sim_require_finite=True,
                sim_require_nnan=True,
                nc=nc,
            )
        )

    devices = jax.devices()[:NCORES]
    mesh = Mesh(np.asarray(devices), ("core",))
    n_outs = len(out_names)
    sharded = jax.jit(
        shard_map(
            _body,
            mesh=mesh,
            in_specs=(PartitionSpec("core"),) * (n_params + n_outs),
            out_specs=(PartitionSpec("core"),) * n_outs,
            check_rep=False,
        ),
        keep_unused=True,
    )

    def run(in_maps):
        concat_in = [
            np.concatenate([np.asarray(in_maps[c][nm]) for c in range(NCORES)], axis=0)
            for nm in in_names
        ]
        concat_zeros = [
            np.zeros((NCORES * shape[0], *shape[1:]), dtype)
            for shape, dtype in out_shapes
        ]
        outs = sharded(*concat_in, *concat_zeros)
        return [
            {
                nm: np.asarray(outs[i]).reshape(NCORES, *out_shapes[i][0])[c]
                for i, nm in enumerate(out_names)
            }
            for c in range(NCORES)
        ]

    _CACHE[key] = run
    return run


def kernel(embed, mask, w_q, w_k, w_v, w_o):
    embed = np.asarray(embed, np.float32)
    mask = np.asarray(mask, bool)
    w_q = np.asarray(w_q, np.float32)
    w_k = np.asarray(w_k, np.float32)
    w_v = np.asarray(w_v, np.float32)
    w_o = np.asarray(w_o, np.float32)

    tril = np.tril(np.ones((S, S), dtype=bool))
    if all(np.array_equal(mask[b], tril) for b in range(B)):
        causal = True
    elif mask.all():
        causal = False
    else:
        return _numpy_fallback(embed, mask, w_q, w_k, w_v, w_o)

    run = _get_runner(causal)
    in_maps = _host_inputs(embed, w_q, w_k, w_v, w_o)
    results = run(in_maps)
    out = np.zeros((B, S, DM), np.float32)
    for b in range(B):
        out[b] = results[2 * b]["out"] + results[2 * b + 1]["out"]
    return out


# revision 6
# speedup vs baseline: 1.0630x; 1.0630x over previous
"""Multi-head attention (B=4, S=2048, DM=1024, H=16, DH=64) on 8 TRN2 cores.

Sharding: 8 cores = 4 batches x 2 head-halves. Core c handles batch c//2 and
heads [ (c%2)*8, (c%2)*8+8 ).  Each core projects Q/K/V for its 8 heads,
runs causal softmax attention, applies its slice of w_o, and writes a partial
[S, DM] output.  The host sums the two partials per batch.

Attention layout (v2): logits are computed transposed ([kv, q]) per head pair,
exponentiated into an SBUF buffer, and the PV product runs in the *flipped*
orientation -- exp tile as the stationary operand, V (with a fused ones column
for the softmax denominator) as the moving operand -- so each PV matmul
streams only 65 columns instead of 512.  The resulting [q, head-dim] tiles
are normalized with per-partition reciprocal scalars, transposed back to
[head-dim, q] on the tensor engine, and fed to the w_o projection.

All matmuls run in bf16 with fp32 PSUM accumulation; logits skip row-max
subtraction (inputs are O(1) so exp cannot overflow).
"""

import math

import ml_dtypes
import numpy as np

B, S, DM, H, DH = 4, 2048, 1024, 16, 64
NCORES = 8
HPC = H // 2        # heads per core
PAIRS = HPC // 2    # head pairs per core (packed 2-per-128-partitions)
F = 512             # query block (free dim of QK matmuls)
CH = 128            # kv chunk (partition dim of transposed logits)
NQB = S // F        # query blocks
NT = S // CH        # kv chunks
VE = DH + 1         # V extended with a ones column (fused denominator)
KT = DM // 128      # contraction k-tiles for projections
KO = HPC * DH // 128  # contraction k-tiles for w_o
NST = S // CH       # query sub-tiles of 128 (same granularity as kv chunks)
SCALE = 1.0 / math.sqrt(DH)

_CACHE = {}


def _split_excess_waits(nc):
    """This environment's walrus rejects instructions carrying more than one
    sync wait ("Too many sync wait commands").  Hoist excess waits onto
    single-wait NoOps inserted right before the offending instruction."""
    import concourse.mybir as mybir

    n = 0
    for f in nc.m.functions:
        for blk in f.blocks:
            newlist = []
            for ins in blk.instructions:
                si = ins.sync_info
                if si is not None and len(si.on_wait) > 1:
                    for w in si.on_wait[:-1]:
                        n += 1
                        newlist.append(
                            mybir.InstNoOp(
                                name=f"I-waitfix-{n}",
                                opcode="NoOp",
                                engine=ins.engine,
                                sync_info=mybir.SyncInfo(on_wait=[w], on_update=[]),
                            )
                        )
                    si.on_wait = si.on_wait[-1:]
                newlist.append(ins)
            blk.instructions = newlist
    return n


def _build(causal, reps=1):
    import concourse.bass as bass
    import concourse.mybir as mybir
    import concourse.tile as tile

    bf16 = mybir.dt.bfloat16
    f32 = mybir.dt.float32
    Exp = mybir.ActivationFunctionType.Exp

    nc = bass.Bass()
    et = nc.dram_tensor("et", [DM, S], bf16, kind="ExternalInput")
    wq = nc.dram_tensor("wq", [DM, HPC * DH], bf16, kind="ExternalInput")
    wk = nc.dram_tensor("wk", [DM, HPC * DH], bf16, kind="ExternalInput")
    wv = nc.dram_tensor("wv", [DM, HPC * DH], bf16, kind="ExternalInput")
    wo = nc.dram_tensor("wo", [HPC * DH, DM], bf16, kind="ExternalInput")
    band = nc.dram_tensor("band", [CH, 2 * F], bf16, kind="ExternalInput")
    ident = nc.dram_tensor("ident", [128, 128], f32, kind="ExternalInput")
    out = nc.dram_tensor("out", [S, DM], f32, kind="ExternalOutput")

    with tile.TileContext(nc) as tc:
        with tc.tile_pool(name="const", bufs=1) as cpool, \
             tc.tile_pool(name="qk", bufs=3) as qkpool, \
             tc.tile_pool(name="eexp", bufs=24) as epool, \
             tc.tile_pool(name="hn", bufs=3) as hnpool, \
             tc.tile_pool(name="outp", bufs=2) as opool, \
             tc.tile_pool(name="small", bufs=3) as spool, \
             tc.tile_pool(name="ps", bufs=1, space="PSUM") as ps:

            # ---- constant loads (few large DMAs; order matters for startup)
            wv_t = cpool.tile([128, KT * HPC * DH], bf16, name="wv_t")
            nc.sync.dma_start(
                wv_t.rearrange("p (a n) -> p a n", a=KT),
                wv.rearrange("(a p) n -> p a n", p=128),
            )
            et_t = cpool.tile([128, KT * S], bf16, name="et_t")
            NQ4 = S // 4
            et_t3 = et_t.rearrange("p (a s) -> p a s", a=KT)
            et3 = et.rearrange("(a p) s -> p a s", p=128)
            # first V chunk's columns land first so compute starts early
            nc.sync.dma_start(et_t3[:, :, 0:CH], et3[:, :, 0:CH])
            nc.sync.dma_start(et_t3[:, :, CH:NQ4], et3[:, :, CH:NQ4])
            w_tiles = {}
            for nm, src in (("wk", wk), ("wq", wq)):
                t = cpool.tile([128, KT * HPC * DH], bf16, name=f"{nm}_t")
                nc.sync.dma_start(
                    t.rearrange("p (a n) -> p a n", a=KT),
                    src.rearrange("(a p) n -> p a n", p=128),
                )
                w_tiles[nm] = t
            wq_t, wk_t = w_tiles["wq"], w_tiles["wk"]
            for cq in range(1, 4):
                nc.sync.dma_start(
                    et_t3[:, :, cq * NQ4 : (cq + 1) * NQ4],
                    et3[:, :, cq * NQ4 : (cq + 1) * NQ4],
                )
            wo_t = cpool.tile([128, KO * DM], bf16, name="wo_t")
            nc.sync.dma_start(
                wo_t.rearrange("p (a n) -> p a n", a=KO),
                wo.rearrange("(a p) n -> p a n", p=128),
            )
            band_t = cpool.tile([CH, 2 * F], bf16, name="band_t")
            nc.sync.dma_start(band_t[:], band[:])
            ident_t = cpool.tile([128, 128], f32, name="ident_t")
            nc.sync.dma_start(ident_t[:], ident[:])

            for _rep in range(reps):
                # V projection target: per kv chunk i, 8 heads x (64 + ones)
                vsb = cpool.tile([128, NT * HPC * VE], bf16, name="vsb")
                nc.vector.memset(
                    vsb.rearrange("p (i e) -> p i e", e=VE)[:, :, DH:VE], 1.0
                )
                headsT = [
                    cpool.tile([128, S], bf16, name=f"headsT{t}", tag=f"headsT{t}")
                    for t in range(PAIRS)
                ]

                def vblock(i):
                    """V projection for kv chunk i (all 8 heads)."""
                    vps = ps.tile([128, 512], f32, tag="mm512", bufs=2, name="vps")
                    for kt in range(KT):
                        nc.tensor.matmul(
                            vps[:],
                            et_t[:, kt * S + i * CH : kt * S + (i + 1) * CH],
                            wv_t[:, kt * HPC * DH : (kt + 1) * HPC * DH],
                            start=(kt == 0),
                            stop=(kt == KT - 1),
                        )
                    nc.vector.tensor_copy(
                        vsb[:, i * HPC * VE : (i + 1) * HPC * VE].rearrange(
                            "p (h e) -> p h e", e=VE
                        )[:, :, 0:DH],
                        vps.rearrange("p (h d) -> p h d", d=DH),
                    )

                qk_tiles = {}

                def qkblock(p, which, j):
                    """Q or K projection for pair p, query block j -> [hd, q]."""
                    key = (p, which)
                    if key not in qk_tiles:
                        qk_tiles[key] = qkpool.tile(
                            [128, S], bf16, tag=f"{which}t2", name=f"{which}t2"
                        )
                    dst = qk_tiles[key]
                    wt = wq_t if which == "q" else wk_t
                    pps = ps.tile([128, 512], f32, tag="mm512", bufs=2, name="pps")
                    for kt in range(KT):
                        nc.tensor.matmul(
                            pps[:],
                            wt[:, kt * HPC * DH + p * 128 : kt * HPC * DH + (p + 1) * 128],
                            et_t[:, kt * S + j * F : kt * S + (j + 1) * F],
                            start=(kt == 0),
                            stop=(kt == KT - 1),
                        )
                    nc.vector.tensor_copy(dst[:, j * F : (j + 1) * F], pps[:])
                    return dst

                def wo_block(st):
                    """Output projection for query sub-tile st (128 queries)."""
                    for nh in range(2):
                        ot = opool.tile([128, 512], f32, tag="ot", name="ot")
                        wps = ps.tile([128, 512], f32, tag="mm512", bufs=2, name="wps")
                        for ktt in range(KO):
                            nc.tensor.matmul(
                                wps[:],
                                headsT[ktt][:, st * CH : (st + 1) * CH],
                                wo_t[:, ktt * DM + nh * 512 : ktt * DM + (nh + 1) * 512],
                                start=(ktt == 0),
                                stop=(ktt == KO - 1),
                            )
                        nc.vector.tensor_copy(ot[:], wps[:])
                        nc.sync.dma_start(
                            out[st * CH : (st + 1) * CH, nh * 512 : (nh + 1) * 512],
                            ot[:],
                        )

                # startup: just enough projections for pair 0 / PV of qb 0
                qkblock(0, "k", 0)
                qkblock(0, "q", 0)
                for i in range(4):
                    vblock(i)

                for p in range(PAIRS):
                    qt2 = qk_tiles[(p, "q")]
                    kt2 = qk_tiles[(p, "k")]
                    qb_order = (
                        (1, 2, 3, 0) if p == PAIRS - 1 and causal else range(NQB)
                    )

                    for qbi, qb in enumerate(qb_order):
                        nch = 4 * qb + 4 if causal else NT

                        # r0: first causally-live query column within this
                        # block for chunk c (narrowed band computation)
                        def _r0(c):
                            return (c - 4 * qb) * CH if causal and c >= 4 * qb else 0

                        # ---- QK + exp per kv chunk (per-chunk e tiles)
                        e_tiles = []
                        for c in range(nch):
                            r0 = _r0(c)
                            stg = ps.tile([128, 2 * F], f32, tag="stg", bufs=2, name="stg")
                            ec = epool.tile([128, 2 * F], bf16, tag="e", name="ec")
                            e_tiles.append(ec)
                            for hh in (0, 1):
                                nc.tensor.matmul(
                                    stg[:, hh * F + r0 : (hh + 1) * F],
                                    kt2[64 * hh : 64 * hh + 64, c * CH : (c + 1) * CH],
                                    qt2[64 * hh : 64 * hh + 64, qb * F + r0 : (qb + 1) * F],
                                    start=True,
                                    stop=True,
                                )
                            st3 = stg.rearrange("p (h f) -> p h f", h=2)[:, :, r0:F]
                            ex3 = ec.rearrange("p (h f) -> p h f", h=2)[:, :, r0:F]
                            nc.scalar.activation(ex3, st3, Exp, scale=SCALE)
                            if causal and c >= 4 * qb:
                                for hh in (0, 1):
                                    sl = ec[:, hh * F + r0 : (hh + 1) * F]
                                    nc.vector.tensor_mul(
                                        sl, sl, band_t[:, F : 2 * F - r0]
                                    )

                        # ---- flipped PV per query sub-tile of 128, then
                        # normalize, transpose back, and (last pair) w_o
                        for stl in range(F // CH):
                            st = qb * (F // CH) + stl
                            ncv = 4 * qb + stl + 1 if causal else NT
                            # pv holds the PV accumulators in cols 0:130 and
                            # the transposed result in cols 256:384 (same
                            # PSUM bank; disjoint regions)
                            pv = ps.tile([128, 512], f32, tag="pv", bufs=2, name="pv")
                            for c in range(ncv):
                                for hh in (0, 1):
                                    nc.tensor.matmul(
                                        pv[:, hh * VE : (hh + 1) * VE],
                                        e_tiles[c][:, hh * F + stl * CH : hh * F + (stl + 1) * CH],
                                        vsb[:, c * HPC * VE + (2 * p + hh) * VE : c * HPC * VE + (2 * p + hh + 1) * VE],
                                        start=(c == 0),
                                        stop=(c == ncv - 1),
                                    )
                            pv3 = pv[:, 0 : 2 * VE].rearrange("p (h e) -> p h e", e=VE)
                            rec = spool.tile([128, 2], f32, tag="rec", name="rec")
                            hn = hnpool.tile([128, 128], f32, tag="hn", name="hn")
                            nc.vector.reciprocal(rec[:], pv3[:, :, DH : DH + 1])
                            for hh in (0, 1):
                                nc.vector.tensor_scalar_mul(
                                    hn[:, hh * DH : (hh + 1) * DH],
                                    pv3[:, hh, 0:DH],
                                    rec[:, hh : hh + 1],
                                )
                            nc.tensor.transpose(pv[:, 256:384], hn[:], ident_t[:])
                            nc.vector.tensor_copy(
                                headsT[p][:, st * CH : (st + 1) * CH], pv[:, 256:384]
                            )
                            if p == PAIRS - 1:
                                wo_block(st)

                        # ---- fillers: projections for the next pair / V
                        if p == 0 and causal and qbi < NQB - 1:
                            for i in range(4 * qbi + 4, 4 * qbi + 8):
                                vblock(i)
                        if p < PAIRS - 1:
                            qkblock(p + 1, "k", qbi)
                            qkblock(p + 1, "q", qbi)
                    if p == 0 and causal:
                        for i in range(12, 16):
                            vblock(i)
                    if not causal and p == 0:
                        for i in range(4, NT):
                            vblock(i)

    _split_excess_waits(nc)
    return nc


def _get_nc(causal):
    key = ("nc", causal)
    if key not in _CACHE:
        _CACHE[key] = _build(causal)
    return _CACHE[key]


def _host_inputs(embed, w_q, w_k, w_v, w_o):
    """Per-core input dicts (bf16 pre-cast / pre-transposed on host)."""
    bf = ml_dtypes.bfloat16
    band = (np.arange(CH)[:, None] <= np.arange(2 * F)[None, :] - F).astype(bf)
    ident = np.eye(128, dtype=np.float32)
    ins = []
    for c in range(NCORES):
        b, half = divmod(c, 2)
        h0 = half * HPC
        ins.append(
            {
                "et": np.ascontiguousarray(embed[b].T).astype(bf),
                "wq": np.ascontiguousarray(
                    w_q[h0 : h0 + HPC].transpose(1, 0, 2).reshape(DM, HPC * DH)
                ).astype(bf),
                "wk": np.ascontiguousarray(
                    w_k[h0 : h0 + HPC].transpose(1, 0, 2).reshape(DM, HPC * DH)
                ).astype(bf),
                "wv": np.ascontiguousarray(
                    w_v[h0 : h0 + HPC].transpose(1, 0, 2).reshape(DM, HPC * DH)
                ).astype(bf),
                "wo": np.ascontiguousarray(w_o[h0 * DH : (h0 + HPC) * DH]).astype(bf),
                "band": band,
                "ident": ident,
            }
        )
    return ins


def _numpy_fallback(embed, mask, w_q, w_k, w_v, w_o):
    """Exact fp32 host computation for mask patterns the device kernel does
    not implement (never hit for the reference's causal mask)."""
    out = np.zeros((B, S, DM), np.float32)
    for b in range(B):
        heads = np.zeros((S, H * DH), np.float32)
        for h in range(H):
            q = embed[b] @ w_q[h]
            k = embed[b] @ w_k[h]
            v = embed[b] @ w_v[h]
            logits = (q @ k.T) * SCALE
            logits = np.where(mask[b], logits, -np.inf)
            logits -= logits.max(axis=-1, keepdims=True)
            p = np.exp(logits)
            p /= p.sum(axis=-1, keepdims=True)
            heads[:, h * DH : (h + 1) * DH] = p @ v
        out[b] = heads @ w_o
    return out


def _get_runner(causal):
    """Cached jitted sharded executor for the built Bass module.

    Mirrors bass2jax.run_bass_via_pjrt's multi-core path, but keeps the
    jitted callable so repeated kernel() calls skip re-tracing/compiling."""
    key = ("runner", causal)
    if key in _CACHE:
        return _CACHE[key]

    import jax
    from jax.experimental.shard_map import shard_map
    from jax.sharding import Mesh, PartitionSpec

    import concourse.mybir as mybir
    from concourse import bass2jax

    bass2jax.install_neuronx_cc_hook()
    nc = _get_nc(causal)
    partition_name = nc.partition_id_tensor.name if nc.partition_id_tensor else None
    in_names, out_names, out_avals, out_shapes = [], [], [], []
    for alloc in nc.m.functions[0].allocations:
        if not isinstance(alloc, mybir.MemoryLocationSet):
            continue
        name = alloc.memorylocations[0].name
        if alloc.kind == "ExternalInput":
            if name != partition_name:
                in_names.append(name)
        elif alloc.kind == "ExternalOutput":
            shape = tuple(alloc.tensor_shape)
            dtype = mybir.dt.np(alloc.dtype)
            out_names.append(name)
            out_avals.append(jax.core.ShapedArray(shape, dtype))
            out_shapes.append((shape, dtype))
    n_params = len(in_names)
    all_in_names = list(in_names) + list(out_names)
    if partition_name is not None:
        all_in_names.append(partition_name)

    def _body(*args):
        operands = list(args)
        if partition_name is not None:
            operands.append(bass2jax.partition_id_tensor())
        return tuple(
            bass2jax._bass_exec_p.bind(
                *operands,
                out_avals=tuple(out_avals),
                in_names=tuple(all_in_names),
                out_names=tuple(out_names),
                lowering_input_output_aliases=(),
                sim_require_finite=True,
                sim_require_nnan=True,
                nc=nc,
            )
        )

    devices = jax.devices()[:NCORES]
    mesh = Mesh(np.asarray(devices), ("core",))
    n_outs = len(out_names)
    sharded = jax.jit(
        shard_map(
            _body,
            mesh=mesh,
            in_specs=(PartitionSpec("core"),) * (n_params + n_outs),
            out_specs=(PartitionSpec("core"),) * n_outs,
            check_rep=False,
        ),
        keep_unused=True,
    )

    def run(in_maps):
        concat_in = [
            np.concatenate([np.asarray(in_maps[c][nm]) for c in range(NCORES)], axis=0)
            for nm in in_names
        ]
        concat_zeros = [
            np.zeros((NCORES * shape[0], *shape[1:]), dtype)
            for shape, dtype in out_shapes
        ]
        outs = sharded(*concat_in, *concat_zeros)
        return [
            {
                nm: np.asarray(outs[i]).reshape(NCORES, *out_shapes[i][0])[c]
                for i, nm in enumerate(out_names)
            }
            for c in range(NCORES)
        ]

    _CACHE[key] = run
    return run


def kernel(embed, mask, w_q, w_k, w_v, w_o):
    embed = np.asarray(embed, np.float32)
    mask = np.asarray(mask, bool)
    w_q = np.asarray(w_q, np.float32)
    w_k = np.asarray(w_k, np.float32)
    w_v = np.asarray(w_v, np.float32)
    w_o = np.asarray(w_o, np.float32)

    tril = np.tril(np.ones((S, S), dtype=bool))
    if all(np.array_equal(mask[b], tril) for b in range(B)):
        causal = True
    elif mask.all():
        causal = False
    else:
        return _numpy_fallback(embed, mask, w_q, w_k, w_v, w_o)

    run = _get_runner(causal)
    in_maps = _host_inputs(embed, w_q, w_k, w_v, w_o)
    results = run(in_maps)
    out = np.zeros((B, S, DM), np.float32)
    for b in range(B):
        out[b] = results[2 * b]["out"] + results[2 * b + 1]["out"]
    return out


# revision 10
# speedup vs baseline: 1.1179x; 1.0516x over previous
"""Multi-head attention (B=4, S=2048, DM=1024, H=16, DH=64) on 8 TRN2 cores.

Sharding: 8 cores = 4 batches x 2 head-halves. Core c handles batch c//2 and
heads [ (c%2)*8, (c%2)*8+8 ).  Each core projects Q/K/V for its 8 heads,
runs causal softmax attention, applies its slice of w_o, and writes a partial
[S, DM] output.  The host sums the two partials per batch.

Attention layout (v2): logits are computed transposed ([kv, q]) per head pair,
exponentiated into an SBUF buffer, and the PV product runs in the *flipped*
orientation -- exp tile as the stationary operand, V (with a fused ones column
for the softmax denominator) as the moving operand -- so each PV matmul
streams only 65 columns instead of 512.  The resulting [q, head-dim] tiles
are normalized with per-partition reciprocal scalars, transposed back to
[head-dim, q] on the tensor engine, and fed to the w_o projection.

All matmuls run in bf16 with fp32 PSUM accumulation; logits skip row-max
subtraction (inputs are O(1) so exp cannot overflow).
"""

import math
from collections import deque

import ml_dtypes
import numpy as np

B, S, DM, H, DH = 4, 2048, 1024, 16, 64
NCORES = 8
HPC = H // 2        # heads per core
PAIRS = HPC // 2    # head pairs per core (packed 2-per-128-partitions)
F = 512             # query block (free dim of QK matmuls)
CH = 128            # kv chunk (partition dim of transposed logits)
NQB = S // F        # query blocks
NT = S // CH        # kv chunks
VE = DH + 1         # V extended with a ones column (fused denominator)
KT = DM // 128      # contraction k-tiles for projections
KO = HPC * DH // 128  # contraction k-tiles for w_o
NST = S // CH       # query sub-tiles of 128 (same granularity as kv chunks)
SCALE = 1.0 / math.sqrt(DH)

_CACHE = {}


def _split_excess_waits(nc):
    """This environment's walrus rejects instructions carrying more than one
    sync wait ("Too many sync wait commands").  Hoist excess waits onto
    single-wait NoOps inserted right before the offending instruction."""
    import concourse.mybir as mybir

    n = 0
    for f in nc.m.functions:
        for blk in f.blocks:
            newlist = []
            for ins in blk.instructions:
                si = ins.sync_info
                if si is not None and len(si.on_wait) > 1:
                    for w in si.on_wait[:-1]:
                        n += 1
                        newlist.append(
                            mybir.InstNoOp(
                                name=f"I-waitfix-{n}",
                                opcode="NoOp",
                                engine=ins.engine,
                                sync_info=mybir.SyncInfo(on_wait=[w], on_update=[]),
                            )
                        )
                    si.on_wait = si.on_wait[-1:]
                newlist.append(ins)
            blk.instructions = newlist
    return n


def _build(causal, reps=1):
    import concourse.bass as bass
    import concourse.mybir as mybir
    import concourse.tile as tile

    bf16 = mybir.dt.bfloat16
    f32 = mybir.dt.float32
    Exp = mybir.ActivationFunctionType.Exp

    nc = bass.Bass()
    et = nc.dram_tensor("et", [DM, S], bf16, kind="ExternalInput")
    wq = nc.dram_tensor("wq", [DM, HPC * DH], bf16, kind="ExternalInput")
    wk = nc.dram_tensor("wk", [DM, HPC * DH], bf16, kind="ExternalInput")
    wv = nc.dram_tensor("wv", [DM, HPC * DH], bf16, kind="ExternalInput")
    wo = nc.dram_tensor("wo", [HPC * DH, DM], bf16, kind="ExternalInput")
    band = nc.dram_tensor("band", [CH, 2 * F], bf16, kind="ExternalInput")
    ident = nc.dram_tensor("ident", [128, 128], f32, kind="ExternalInput")
    out = nc.dram_tensor("out", [S, DM], f32, kind="ExternalOutput")

    with tile.TileContext(nc) as tc:
        with tc.tile_pool(name="const", bufs=1) as cpool, \
             tc.tile_pool(name="qk", bufs=3) as qkpool, \
             tc.tile_pool(name="eexp", bufs=24) as epool, \
             tc.tile_pool(name="hn", bufs=3) as hnpool, \
             tc.tile_pool(name="outp", bufs=2) as opool, \
             tc.tile_pool(name="small", bufs=3) as spool, \
             tc.tile_pool(name="ps", bufs=1, space="PSUM") as ps:

            # ---- constant loads (few large DMAs; order matters for startup)
            wv_t = cpool.tile([128, KT * HPC * DH], bf16, name="wv_t")
            nc.sync.dma_start(
                wv_t.rearrange("p (a n) -> p a n", a=KT),
                wv.rearrange("(a p) n -> p a n", p=128),
            )
            et_t = cpool.tile([128, KT * S], bf16, name="et_t")
            NQ4 = S // 4
            et_t3 = et_t.rearrange("p (a s) -> p a s", a=KT)
            et3 = et.rearrange("(a p) s -> p a s", p=128)
            # first V chunks' columns land first so compute starts early
            # (256-col slabs keep the 512B/descriptor fast path)
            nc.sync.dma_start(et_t3[:, :, 0 : 2 * CH], et3[:, :, 0 : 2 * CH])
            nc.sync.dma_start(et_t3[:, :, 2 * CH : NQ4], et3[:, :, 2 * CH : NQ4])
            w_tiles = {}
            for nm, src in (("wk", wk), ("wq", wq)):
                t = cpool.tile([128, KT * HPC * DH], bf16, name=f"{nm}_t")
                nc.sync.dma_start(
                    t.rearrange("p (a n) -> p a n", a=KT),
                    src.rearrange("(a p) n -> p a n", p=128),
                )
                w_tiles[nm] = t
            wq_t, wk_t = w_tiles["wq"], w_tiles["wk"]
            for cq in range(1, 4):
                nc.sync.dma_start(
                    et_t3[:, :, cq * NQ4 : (cq + 1) * NQ4],
                    et3[:, :, cq * NQ4 : (cq + 1) * NQ4],
                )
            wo_t = cpool.tile([128, KO * DM], bf16, name="wo_t")
            nc.sync.dma_start(
                wo_t.rearrange("p (a n) -> p a n", a=KO),
                wo.rearrange("(a p) n -> p a n", p=128),
            )
            band_t = cpool.tile([CH, 2 * F], bf16, name="band_t")
            nc.sync.dma_start(band_t[:], band[:])
            ident_t = cpool.tile([128, 128], f32, name="ident_t")
            nc.sync.dma_start(ident_t[:], ident[:])

            for _rep in range(reps):
                # V projection target: per kv chunk i, 8 heads x (64 + ones)
                vsb = cpool.tile([128, NT * HPC * VE], bf16, name="vsb")
                nc.vector.memset(
                    vsb.rearrange("p (i e) -> p i e", e=VE)[:, :, DH:VE], 1.0
                )
                headsT = [
                    cpool.tile([128, S], bf16, name=f"headsT{t}", tag=f"headsT{t}")
                    for t in range(PAIRS)
                ]

                def vblock(i):
                    """V projection for kv chunk i (all 8 heads)."""
                    vps = ps.tile([128, 512], f32, tag="mm512", bufs=2, name="vps")
                    for kt in range(KT):
                        nc.tensor.matmul(
                            vps[:],
                            et_t[:, kt * S + i * CH : kt * S + (i + 1) * CH],
                            wv_t[:, kt * HPC * DH : (kt + 1) * HPC * DH],
                            start=(kt == 0),
                            stop=(kt == KT - 1),
                        )
                    nc.vector.tensor_copy(
                        vsb[:, i * HPC * VE : (i + 1) * HPC * VE].rearrange(
                            "p (h e) -> p h e", e=VE
                        )[:, :, 0:DH],
                        vps.rearrange("p (h d) -> p h d", d=DH),
                    )

                qk_tiles = {}

                def qkblock(p, which, j):
                    """Q or K projection for pair p, query block j -> [hd, q]."""
                    key = (p, which)
                    if key not in qk_tiles:
                        qk_tiles[key] = qkpool.tile(
                            [128, S], bf16, tag=f"{which}t2", name=f"{which}t2"
                        )
                    dst = qk_tiles[key]
                    wt = wq_t if which == "q" else wk_t
                    pps = ps.tile([128, 512], f32, tag="mm512", bufs=2, name="pps")
                    for kt in range(KT):
                        nc.tensor.matmul(
                            pps[:],
                            wt[:, kt * HPC * DH + p * 128 : kt * HPC * DH + (p + 1) * 128],
                            et_t[:, kt * S + j * F : kt * S + (j + 1) * F],
                            start=(kt == 0),
                            stop=(kt == KT - 1),
                        )
                    nc.vector.tensor_copy(dst[:, j * F : (j + 1) * F], pps[:])
                    return dst

                def wo_block(st):
                    """Output projection for query sub-tile st (128 queries)."""
                    for nh in range(2):
                        ot = opool.tile([128, 512], f32, tag="ot", name="ot")
                        wps = ps.tile([128, 512], f32, tag="mm512", bufs=2, name="wps")
                        for ktt in range(KO):
                            nc.tensor.matmul(
                                wps[:],
                                headsT[ktt][:, st * CH : (st + 1) * CH],
                                wo_t[:, ktt * DM + nh * 512 : ktt * DM + (nh + 1) * 512],
                                start=(ktt == 0),
                                stop=(ktt == KO - 1),
                            )
                        nc.vector.tensor_copy(ot[:], wps[:])
                        nc.sync.dma_start(
                            out[st * CH : (st + 1) * CH, nh * 512 : (nh + 1) * 512],
                            ot[:],
                        )

                # ---- emission bookkeeping: running per-engine cost estimates
                # drive fine-grained interleaving of "filler" PE work (V / QK
                # projections) into the exp-paced attention stream.
                state = {"pe": 0.0, "act": 0.0}

                def note_pe(cycles):
                    state["pe"] += cycles * 0.4167

                def note_act(cols):
                    state["act"] += cols * 0.833 + 185.0

                fill_q = deque()
                fill_done = set()

                def _fill_one():
                    key, fn = fill_q.popleft()
                    fill_done.add(key)
                    fn()

                def fill(slack=1500.0):
                    while fill_q and state["pe"] < state["act"] + slack:
                        _fill_one()

                def force(key):
                    while key not in fill_done:
                        _fill_one()

                def v_f(i):
                    return ("v", i), lambda: (vblock(i), note_pe(KT * 512 + 80))

                def qk_f(p, which, j):
                    return (
                        ("qk", p, which, j),
                        lambda: (qkblock(p, which, j), note_pe(KT * 512 + 80)),
                    )

                # startup: V chunks 0-3 first (their DMAs land first), then
                # pair-0 projections; everything else queued as filler.
                for i in range(4):
                    vblock(i)
                qkblock(0, "k", 0)
                qkblock(0, "q", 0)
                fill_done.update({("v", i) for i in range(4)})
                fill_done.update({("qk", 0, "k", 0), ("qk", 0, "q", 0)})
                for i in range(4, NT):
                    fill_q.append(v_f(i))
                for j in range(1, NQB):
                    fill_q.append(qk_f(0, "k", j))
                    fill_q.append(qk_f(0, "q", j))
                for p in range(1, PAIRS):
                    for j in range(NQB):
                        fill_q.append(qk_f(p, "k", j))
                        fill_q.append(qk_f(p, "q", j))

                pending = deque()  # st-group closures awaiting emission

                def drain_pending(n):
                    for _ in range(min(n, len(pending))):
                        pending.popleft()()

                for p in range(PAIRS):
                    qb_order = (
                        (1, 2, 3, 0) if p == PAIRS - 1 and causal else range(NQB)
                    )

                    for qb in qb_order:
                        nch = 4 * qb + 4 if causal else NT
                        force(("qk", p, "k", qb))
                        force(("qk", p, "q", qb))
                        qt2 = qk_tiles[(p, "q")]
                        kt2 = qk_tiles[(p, "k")]

                        # r0: first causally-live query column within this
                        # block for chunk c (narrowed band computation)
                        def _r0(c):
                            return (c - 4 * qb) * CH if causal and c >= 4 * qb else 0

                        # ---- QK + exp per kv chunk (per-chunk e tiles),
                        # draining previous block's PV groups in between
                        npend = len(pending)
                        e_tiles = []
                        for c in range(nch):
                            r0 = _r0(c)
                            stg = ps.tile([128, 2 * F], f32, tag="stg", bufs=2, name="stg")
                            ec = epool.tile([128, 2 * F], bf16, tag="e", name="ec")
                            e_tiles.append(ec)
                            for hh in (0, 1):
                                nc.tensor.matmul(
                                    stg[:, hh * F + r0 : (hh + 1) * F],
                                    kt2[64 * hh : 64 * hh + 64, c * CH : (c + 1) * CH],
                                    qt2[64 * hh : 64 * hh + 64, qb * F + r0 : (qb + 1) * F],
                                    start=True,
                                    stop=True,
                                )
                            note_pe(2 * (F - r0))
                            st3 = stg.rearrange("p (h f) -> p h f", h=2)[:, :, r0:F]
                            ex3 = ec.rearrange("p (h f) -> p h f", h=2)[:, :, r0:F]
                            nc.scalar.activation(ex3, st3, Exp, scale=SCALE)
                            note_act(2 * (F - r0))
                            if causal and c >= 4 * qb:
                                for hh in (0, 1):
                                    sl = ec[:, hh * F + r0 : (hh + 1) * F]
                                    nc.vector.tensor_mul(
                                        sl, sl, band_t[:, F : 2 * F - r0]
                                    )
                            drain_pending((npend * (c + 1)) // nch - (npend - len(pending)))
                            fill()

                        def make_st(p, qb, stl, e_tiles):
                            def emit():
                                st = qb * (F // CH) + stl
                                ncv = 4 * qb + stl + 1 if causal else NT
                                # pv: PV accumulators in cols 0:130, transposed
                                # result in cols 256:384 (same PSUM bank)
                                pv = ps.tile([128, 512], f32, tag="pv", bufs=2, name="pv")
                                for c in range(ncv):
                                    for hh in (0, 1):
                                        nc.tensor.matmul(
                                            pv[:, hh * VE : (hh + 1) * VE],
                                            e_tiles[c][:, hh * F + stl * CH : hh * F + (stl + 1) * CH],
                                            vsb[:, c * HPC * VE + (2 * p + hh) * VE : c * HPC * VE + (2 * p + hh + 1) * VE],
                                            start=(c == 0),
                                            stop=(c == ncv - 1),
                                        )
                                note_pe(ncv * 2 * VE + 256)
                                pv3 = pv[:, 0 : 2 * VE].rearrange("p (h e) -> p h e", e=VE)
                                rec = spool.tile([128, 2], f32, tag="rec", name="rec")
                                hn = hnpool.tile([128, 128], f32, tag="hn", name="hn")
                                nc.vector.reciprocal(rec[:], pv3[:, :, DH : DH + 1])
                                for hh in (0, 1):
                                    nc.vector.tensor_scalar_mul(
                                        hn[:, hh * DH : (hh + 1) * DH],
                                        pv3[:, hh, 0:DH],
                                        rec[:, hh : hh + 1],
                                    )
                                nc.tensor.transpose(pv[:, 256:384], hn[:], ident_t[:])
                                nc.vector.tensor_copy(
                                    headsT[p][:, st * CH : (st + 1) * CH], pv[:, 256:384]
                                )
                                if p == PAIRS - 1:
                                    wo_block(st)
                                    note_pe(2 * KO * 512 + 160)

                            return emit

                        for stl in range(F // CH):
                            pending.append(make_st(p, qb, stl, e_tiles))

                # flush: remaining PV groups (last pair's final block), filler
                drain_pending(len(pending))
                while fill_q:
                    _fill_one()

    _split_excess_waits(nc)
    return nc


def _get_nc(causal):
    key = ("nc", causal)
    if key not in _CACHE:
        _CACHE[key] = _build(causal)
    return _CACHE[key]


def _host_inputs(embed, w_q, w_k, w_v, w_o):
    """Per-core input dicts (bf16 pre-cast / pre-transposed on host)."""
    bf = ml_dtypes.bfloat16
    band = (np.arange(CH)[:, None] <= np.arange(2 * F)[None, :] - F).astype(bf)
    ident = np.eye(128, dtype=np.float32)
    ins = []
    for c in range(NCORES):
        b, half = divmod(c, 2)
        h0 = half * HPC
        ins.append(
            {
                "et": np.ascontiguousarray(embed[b].T).astype(bf),
                "wq": np.ascontiguousarray(
                    w_q[h0 : h0 + HPC].transpose(1, 0, 2).reshape(DM, HPC * DH)
                ).astype(bf),
                "wk": np.ascontiguousarray(
                    w_k[h0 : h0 + HPC].transpose(1, 0, 2).reshape(DM, HPC * DH)
                ).astype(bf),
                "wv": np.ascontiguousarray(
                    w_v[h0 : h0 + HPC].transpose(1, 0, 2).reshape(DM, HPC * DH)
                ).astype(bf),
                "wo": np.ascontiguousarray(w_o[h0 * DH : (h0 + HPC) * DH]).astype(bf),
                "band": band,
                "ident": ident,
            }
        )
    return ins


def _numpy_fallback(embed, mask, w_q, w_k, w_v, w_o):
    """Exact fp32 host computation for mask patterns the device kernel does
    not implement (never hit for the reference's causal mask)."""
    out = np.zeros((B, S, DM), np.float32)
    for b in range(B):
        heads = np.zeros((S, H * DH), np.float32)
        for h in range(H):
            q = embed[b] @ w_q[h]
            k = embed[b] @ w_k[h]
            v = embed[b] @ w_v[h]
            logits = (q @ k.T) * SCALE
            logits = np.where(mask[b], logits, -np.inf)
            logits -= logits.max(axis=-1, keepdims=True)
            p = np.exp(logits)
            p /= p.sum(axis=-1, keepdims=True)
            heads[:, h * DH : (h + 1) * DH] = p @ v
        out[b] = heads @ w_o
    return out


def _get_runner(causal):
    """Cached jitted sharded executor for the built Bass module.

    Mirrors bass2jax.run_bass_via_pjrt's multi-core path, but keeps the
    jitted callable so repeated kernel() calls skip re-tracing/compiling."""
    key = ("runner", causal)
    if key in _CACHE:
        return _CACHE[key]

    import jax
    from jax.experimental.shard_map import shard_map
    from jax.sharding import Mesh, PartitionSpec

    import concourse.mybir as mybir
    from concourse import bass2jax

    bass2jax.install_neuronx_cc_hook()
    nc = _get_nc(causal)
    partition_name = nc.partition_id_tensor.name if nc.partition_id_tensor else None
    in_names, out_names, out_avals, out_shapes = [], [], [], []
    for alloc in nc.m.functions[0].allocations:
        if not isinstance(alloc, mybir.MemoryLocationSet):
            continue
        name = alloc.memorylocations[0].name
        if alloc.kind == "ExternalInput":
            if name != partition_name:
                in_names.append(name)
        elif alloc.kind == "ExternalOutput":
            shape = tuple(alloc.tensor_shape)
            dtype = mybir.dt.np(alloc.dtype)
            out_names.append(name)
            out_avals.append(jax.core.ShapedArray(shape, dtype))
            out_shapes.append((shape, dtype))
    n_params = len(in_names)
    all_in_names = list(in_names) + list(out_names)
    if partition_name is not None:
        all_in_names.append(partition_name)

    def _body(*args):
        operands = list(args)
        if partition_name is not None:
            operands.append(bass2jax.partition_id_tensor())
        return tuple(
            bass2jax._bass_exec_p.bind(
                *operands,
                out_avals=tuple(out_avals),
                in_names=tuple(all_in_names),
                out_names=tuple(out_names),
                lowering_input_output_aliases=(),
                sim_require_finite=True,
                sim_require_nnan=True,
                nc=nc,
            )
        )

    devices = jax.devices()[:NCORES]
    mesh = Mesh(np.asarray(devices), ("core",))
    n_outs = len(out_names)
    sharded = jax.jit(
        shard_map(
            _body,
            mesh=mesh,
            in_specs=(PartitionSpec("core"),) * (n_params + n_outs),
            out_specs=(PartitionSpec("core"),) * n_outs,
            check_rep=False,
        ),
        keep_unused=True,
    )

    def run(in_maps):
        concat_in = [
            np.concatenate([np.asarray(in_maps[c][nm]) for c in range(NCORES)], axis=0)
            for nm in in_names
        ]
        concat_zeros = [
            np.zeros((NCORES * shape[0], *shape[1:]), dtype)
            for shape, dtype in out_shapes
        ]
        outs = sharded(*concat_in, *concat_zeros)
        return [
            {
                nm: np.asarray(outs[i]).reshape(NCORES, *out_shapes[i][0])[c]
                for i, nm in enumerate(out_names)
            }
            for c in range(NCORES)
        ]

    _CACHE[key] = run
    return run


def kernel(embed, mask, w_q, w_k, w_v, w_o):
    embed = np.asarray(embed, np.float32)
    mask = np.asarray(mask, bool)
    w_q = np.asarray(w_q, np.float32)
    w_k = np.asarray(w_k, np.float32)
    w_v = np.asarray(w_v, np.float32)
    w_o = np.asarray(w_o, np.float32)

    tril = np.tril(np.ones((S, S), dtype=bool))
    if all(np.array_equal(mask[b], tril) for b in range(B)):
        causal = True
    elif mask.all():
        causal = False
    else:
        return _numpy_fallback(embed, mask, w_q, w_k, w_v, w_o)

    run = _get_runner(causal)
    in_maps = _host_inputs(embed, w_q, w_k, w_v, w_o)
    results = run(in_maps)
    out = np.zeros((B, S, DM), np.float32)
    for b in range(B):
        out[b] = results[2 * b]["out"] + results[2 * b + 1]["out"]
    return out


# revision 11
# speedup vs baseline: 1.1324x; 1.0130x over previous
"""Multi-head attention (B=4, S=2048, DM=1024, H=16, DH=64) on 8 TRN2 cores.

Sharding: 8 cores = 4 batches x 2 head-halves. Core c handles batch c//2 and
heads [ (c%2)*8, (c%2)*8+8 ).  Each core projects Q/K/V for its 8 heads,
runs causal softmax attention, applies its slice of w_o, and writes a partial
[S, DM] output.  The host sums the two partials per batch.

Attention layout (v2): logits are computed transposed ([kv, q]) per head pair,
exponentiated into an SBUF buffer, and the PV product runs in the *flipped*
orientation -- exp tile as the stationary operand, V (with a fused ones column
for the softmax denominator) as the moving operand -- so each PV matmul
streams only 65 columns instead of 512.  The resulting [q, head-dim] tiles
are normalized with per-partition reciprocal scalars, transposed back to
[head-dim, q] on the tensor engine, and fed to the w_o projection.

All matmuls run in bf16 with fp32 PSUM accumulation; logits skip row-max
subtraction (inputs are O(1) so exp cannot overflow).
"""

import math
from collections import deque

import ml_dtypes
import numpy as np

B, S, DM, H, DH = 4, 2048, 1024, 16, 64
NCORES = 8
HPC = H // 2        # heads per core
PAIRS = HPC // 2    # head pairs per core (packed 2-per-128-partitions)
F = 512             # query block (free dim of QK matmuls)
CH = 128            # kv chunk (partition dim of transposed logits)
NQB = S // F        # query blocks
NT = S // CH        # kv chunks
VE = DH + 1         # V extended with a ones column (fused denominator)
KT = DM // 128      # contraction k-tiles for projections
KO = HPC * DH // 128  # contraction k-tiles for w_o
NST = S // CH       # query sub-tiles of 128 (same granularity as kv chunks)
SCALE = 1.0 / math.sqrt(DH)

_CACHE = {}


def _split_excess_waits(nc):
    """This environment's walrus rejects instructions carrying more than one
    sync wait ("Too many sync wait commands").  Hoist excess waits onto
    single-wait NoOps inserted right before the offending instruction."""
    import concourse.mybir as mybir

    n = 0
    for f in nc.m.functions:
        for blk in f.blocks:
            newlist = []
            for ins in blk.instructions:
                si = ins.sync_info
                if si is not None and len(si.on_wait) > 1:
                    for w in si.on_wait[:-1]:
                        n += 1
                        newlist.append(
                            mybir.InstNoOp(
                                name=f"I-waitfix-{n}",
                                opcode="NoOp",
                                engine=ins.engine,
                                sync_info=mybir.SyncInfo(on_wait=[w], on_update=[]),
                            )
                        )
                    si.on_wait = si.on_wait[-1:]
                newlist.append(ins)
            blk.instructions = newlist
    return n


def _build(causal, reps=1):
    import concourse.bass as bass
    import concourse.mybir as mybir
    import concourse.tile as tile

    bf16 = mybir.dt.bfloat16
    f32 = mybir.dt.float32
    Exp = mybir.ActivationFunctionType.Exp

    nc = bass.Bass()
    et = nc.dram_tensor("et", [DM, S], bf16, kind="ExternalInput")
    wq = nc.dram_tensor("wq", [DM, HPC * DH], bf16, kind="ExternalInput")
    wk = nc.dram_tensor("wk", [DM, HPC * DH], bf16, kind="ExternalInput")
    wv = nc.dram_tensor("wv", [DM, HPC * DH], bf16, kind="ExternalInput")
    wo = nc.dram_tensor("wo", [HPC * DH, DM], bf16, kind="ExternalInput")
    band = nc.dram_tensor("band", [CH, 2 * F], bf16, kind="ExternalInput")
    ident = nc.dram_tensor("ident", [128, 128], f32, kind="ExternalInput")
    out = nc.dram_tensor("out", [S, DM], f32, kind="ExternalOutput")

    with tile.TileContext(nc) as tc:
        with tc.tile_pool(name="const", bufs=1) as cpool, \
             tc.tile_pool(name="qk", bufs=3) as qkpool, \
             tc.tile_pool(name="eexp", bufs=24) as epool, \
             tc.tile_pool(name="hn", bufs=3) as hnpool, \
             tc.tile_pool(name="outp", bufs=2) as opool, \
             tc.tile_pool(name="small", bufs=3) as spool, \
             tc.tile_pool(name="ps", bufs=1, space="PSUM") as ps:

            # ---- constant loads (few large DMAs; order matters for startup)
            wv_t = cpool.tile([128, KT * HPC * DH], bf16, name="wv_t")
            nc.sync.dma_start(
                wv_t.rearrange("p (a n) -> p a n", a=KT),
                wv.rearrange("(a p) n -> p a n", p=128),
            )
            et_t = cpool.tile([128, KT * S], bf16, name="et_t")
            NQ4 = S // 4
            et_t3 = et_t.rearrange("p (a s) -> p a s", a=KT)
            et3 = et.rearrange("(a p) s -> p a s", p=128)
            # first V chunks' columns land first so compute starts early
            # (256-col slabs keep the 512B/descriptor fast path)
            nc.sync.dma_start(et_t3[:, :, 0 : 2 * CH], et3[:, :, 0 : 2 * CH])
            nc.sync.dma_start(et_t3[:, :, 2 * CH : NQ4], et3[:, :, 2 * CH : NQ4])
            w_tiles = {}
            for nm, src in (("wk", wk), ("wq", wq)):
                t = cpool.tile([128, KT * HPC * DH], bf16, name=f"{nm}_t")
                nc.sync.dma_start(
                    t.rearrange("p (a n) -> p a n", a=KT),
                    src.rearrange("(a p) n -> p a n", p=128),
                )
                w_tiles[nm] = t
            wq_t, wk_t = w_tiles["wq"], w_tiles["wk"]
            for cq in range(1, 4):
                nc.sync.dma_start(
                    et_t3[:, :, cq * NQ4 : (cq + 1) * NQ4],
                    et3[:, :, cq * NQ4 : (cq + 1) * NQ4],
                )
            wo_t = cpool.tile([128, KO * DM], bf16, name="wo_t")
            nc.sync.dma_start(
                wo_t.rearrange("p (a n) -> p a n", a=KO),
                wo.rearrange("(a p) n -> p a n", p=128),
            )
            band_t = cpool.tile([CH, 2 * F], bf16, name="band_t")
            nc.sync.dma_start(band_t[:], band[:])
            ident_t = cpool.tile([128, 128], f32, name="ident_t")
            nc.sync.dma_start(ident_t[:], ident[:])

            for _rep in range(reps):
                # V projection target: per kv chunk i, 8 heads x (64 + ones)
                vsb = cpool.tile([128, NT * HPC * VE], bf16, name="vsb")
                nc.vector.memset(
                    vsb.rearrange("p (i e) -> p i e", e=VE)[:, :, DH:VE], 1.0
                )
                headsT = [
                    cpool.tile([128, S], bf16, name=f"headsT{t}", tag=f"headsT{t}")
                    for t in range(PAIRS)
                ]

                def vblock(i):
                    """V projection for kv chunk i (all 8 heads)."""
                    vps = ps.tile([128, 512], f32, tag="mm512", bufs=2, name="vps")
                    for kt in range(KT):
                        nc.tensor.matmul(
                            vps[:],
                            et_t[:, kt * S + i * CH : kt * S + (i + 1) * CH],
                            wv_t[:, kt * HPC * DH : (kt + 1) * HPC * DH],
                            start=(kt == 0),
                            stop=(kt == KT - 1),
                        )
                    nc.vector.tensor_copy(
                        vsb[:, i * HPC * VE : (i + 1) * HPC * VE].rearrange(
                            "p (h e) -> p h e", e=VE
                        )[:, :, 0:DH],
                        vps.rearrange("p (h d) -> p h d", d=DH),
                    )

                qk_tiles = {}

                def qkblock(p, which, j):
                    """Q or K projection for pair p, query block j -> [hd, q]."""
                    key = (p, which)
                    if key not in qk_tiles:
                        qk_tiles[key] = qkpool.tile(
                            [128, S], bf16, tag=f"{which}t2", name=f"{which}t2"
                        )
                    dst = qk_tiles[key]
                    wt = wq_t if which == "q" else wk_t
                    pps = ps.tile([128, 512], f32, tag="mm512", bufs=2, name="pps")
                    for kt in range(KT):
                        nc.tensor.matmul(
                            pps[:],
                            wt[:, kt * HPC * DH + p * 128 : kt * HPC * DH + (p + 1) * 128],
                            et_t[:, kt * S + j * F : kt * S + (j + 1) * F],
                            start=(kt == 0),
                            stop=(kt == KT - 1),
                        )
                    nc.vector.tensor_copy(dst[:, j * F : (j + 1) * F], pps[:])
                    return dst

                def wo_block(st):
                    """Output projection for query sub-tile st (128 queries)."""
                    for nh in range(2):
                        ot = opool.tile([128, 512], f32, tag="ot", name="ot")
                        wps = ps.tile([128, 512], f32, tag="mm512", bufs=2, name="wps")
                        for ktt in range(KO):
                            nc.tensor.matmul(
                                wps[:],
                                headsT[ktt][:, st * CH : (st + 1) * CH],
                                wo_t[:, ktt * DM + nh * 512 : ktt * DM + (nh + 1) * 512],
                                start=(ktt == 0),
                                stop=(ktt == KO - 1),
                            )
                        nc.vector.tensor_copy(ot[:], wps[:])
                        nc.sync.dma_start(
                            out[st * CH : (st + 1) * CH, nh * 512 : (nh + 1) * 512],
                            ot[:],
                        )

                # ---- emission bookkeeping: running per-engine cost estimates
                # drive fine-grained interleaving of "filler" PE work (V / QK
                # projections) into the exp-paced attention stream.
                state = {"pe": 0.0, "act": 0.0}

                def note_pe(cycles):
                    state["pe"] += cycles * 0.4167

                def note_act(cols):
                    state["act"] += cols * 0.833 + 185.0

                fill_q = deque()
                fill_done = set()

                def _fill_one():
                    key, fn = fill_q.popleft()
                    fill_done.add(key)
                    fn()

                def fill(slack=1500.0):
                    while fill_q and state["pe"] < state["act"] + slack:
                        _fill_one()

                def force(key):
                    if key in fill_done:
                        return
                    for idx, (k, fn) in enumerate(fill_q):
                        if k == key:
                            del fill_q[idx]
                            fill_done.add(key)
                            fn()
                            return

                def v_f(i):
                    return ("v", i), lambda: (vblock(i), note_pe(KT * 512 + 80))

                def qk_f(p, which, j):
                    return (
                        ("qk", p, which, j),
                        lambda: (qkblock(p, which, j), note_pe(KT * 512 + 80)),
                    )

                # startup: V chunks 0-3 first (their DMAs land first), then
                # pair-0 projections; everything else queued as filler.
                for i in range(4):
                    vblock(i)
                qkblock(0, "k", 0)
                qkblock(0, "q", 0)
                fill_done.update({("v", i) for i in range(4)})
                fill_done.update({("qk", 0, "k", 0), ("qk", 0, "q", 0)})
                for i in range(4, NT):
                    fill_q.append(v_f(i))
                for j in range(1, NQB):
                    fill_q.append(qk_f(0, "k", j))
                    fill_q.append(qk_f(0, "q", j))
                for p in range(1, PAIRS):
                    for j in range(NQB):
                        fill_q.append(qk_f(p, "k", j))
                        fill_q.append(qk_f(p, "q", j))

                pending = deque()  # st-group closures awaiting emission

                def drain_pending(n):
                    for _ in range(min(n, len(pending))):
                        pending.popleft()()

                for p in range(PAIRS):
                    qb_order = (
                        (1, 2, 3, 0) if p == PAIRS - 1 and causal else range(NQB)
                    )

                    for qb in qb_order:
                        nch = 4 * qb + 4 if causal else NT
                        force(("qk", p, "k", qb))
                        force(("qk", p, "q", qb))
                        qt2 = qk_tiles[(p, "q")]
                        kt2 = qk_tiles[(p, "k")]

                        # r0: first causally-live query column within this
                        # block for chunk c (narrowed band computation)
                        def _r0(c):
                            return (c - 4 * qb) * CH if causal and c >= 4 * qb else 0

                        # ---- QK + exp per kv chunk (per-chunk e tiles),
                        # draining previous block's PV groups in between
                        npend = len(pending)
                        e_tiles = []
                        for c in range(nch):
                            r0 = _r0(c)
                            stg = ps.tile([128, 2 * F], f32, tag="stg", bufs=2, name="stg")
                            ec = epool.tile([128, 2 * F], bf16, tag="e", name="ec")
                            e_tiles.append(ec)
                            for hh in (0, 1):
                                nc.tensor.matmul(
                                    stg[:, hh * F + r0 : (hh + 1) * F],
                                    kt2[64 * hh : 64 * hh + 64, c * CH : (c + 1) * CH],
                                    qt2[64 * hh : 64 * hh + 64, qb * F + r0 : (qb + 1) * F],
                                    start=True,
                                    stop=True,
                                )
                            note_pe(2 * (F - r0))
                            st3 = stg.rearrange("p (h f) -> p h f", h=2)[:, :, r0:F]
                            ex3 = ec.rearrange("p (h f) -> p h f", h=2)[:, :, r0:F]
                            nc.scalar.activation(ex3, st3, Exp, scale=SCALE)
                            note_act(2 * (F - r0))
                            if causal and c >= 4 * qb:
                                for hh in (0, 1):
                                    sl = ec[:, hh * F + r0 : (hh + 1) * F]
                                    nc.vector.tensor_mul(
                                        sl, sl, band_t[:, F : 2 * F - r0]
                                    )
                            drain_pending((npend * (c + 1)) // nch - (npend - len(pending)))
                            fill()

                        def make_st(p, qb, stl, e_tiles):
                            def emit():
                                st = qb * (F // CH) + stl
                                ncv = 4 * qb + stl + 1 if causal else NT
                                # pv: PV accumulators in cols 0:130, transposed
                                # result in cols 256:384 (same PSUM bank)
                                pv = ps.tile([128, 512], f32, tag="pv", bufs=2, name="pv")
                                for c in range(ncv):
                                    for hh in (0, 1):
                                        nc.tensor.matmul(
                                            pv[:, hh * VE : (hh + 1) * VE],
                                            e_tiles[c][:, hh * F + stl * CH : hh * F + (stl + 1) * CH],
                                            vsb[:, c * HPC * VE + (2 * p + hh) * VE : c * HPC * VE + (2 * p + hh + 1) * VE],
                                            start=(c == 0),
                                            stop=(c == ncv - 1),
                                        )
                                note_pe(ncv * 2 * VE + 256)
                                pv3 = pv[:, 0 : 2 * VE].rearrange("p (h e) -> p h e", e=VE)
                                rec = spool.tile([128, 2], f32, tag="rec", name="rec")
                                hn = hnpool.tile([128, 128], f32, tag="hn", name="hn")
                                nc.vector.reciprocal(rec[:], pv3[:, :, DH : DH + 1])
                                for hh in (0, 1):
                                    nc.vector.tensor_scalar_mul(
                                        hn[:, hh * DH : (hh + 1) * DH],
                                        pv3[:, hh, 0:DH],
                                        rec[:, hh : hh + 1],
                                    )
                                nc.tensor.transpose(pv[:, 256:384], hn[:], ident_t[:])
                                nc.vector.tensor_copy(
                                    headsT[p][:, st * CH : (st + 1) * CH], pv[:, 256:384]
                                )
                                if p == PAIRS - 1:
                                    wo_block(st)
                                    note_pe(2 * KO * 512 + 160)

                            return emit

                        for stl in range(F // CH):
                            pending.append(make_st(p, qb, stl, e_tiles))

                # flush: remaining PV groups (last pair's final block), filler
                drain_pending(len(pending))
                while fill_q:
                    _fill_one()

    _split_excess_waits(nc)
    return nc


def _get_nc(causal):
    key = ("nc", causal)
    if key not in _CACHE:
        _CACHE[key] = _build(causal)
    return _CACHE[key]


def _host_inputs(embed, w_q, w_k, w_v, w_o):
    """Per-core input dicts (bf16 pre-cast / pre-transposed on host)."""
    bf = ml_dtypes.bfloat16
    band = (np.arange(CH)[:, None] <= np.arange(2 * F)[None, :] - F).astype(bf)
    ident = np.eye(128, dtype=np.float32)
    ins = []
    for c in range(NCORES):
        b, half = divmod(c, 2)
        h0 = half * HPC
        ins.append(
            {
                "et": np.ascontiguousarray(embed[b].T).astype(bf),
                "wq": np.ascontiguousarray(
                    w_q[h0 : h0 + HPC].transpose(1, 0, 2).reshape(DM, HPC * DH)
                ).astype(bf),
                "wk": np.ascontiguousarray(
                    w_k[h0 : h0 + HPC].transpose(1, 0, 2).reshape(DM, HPC * DH)
                ).astype(bf),
                "wv": np.ascontiguousarray(
                    w_v[h0 : h0 + HPC].transpose(1, 0, 2).reshape(DM, HPC * DH)
                ).astype(bf),
                "wo": np.ascontiguousarray(w_o[h0 * DH : (h0 + HPC) * DH]).astype(bf),
                "band": band,
                "ident": ident,
            }
        )
    return ins


def _numpy_fallback(embed, mask, w_q, w_k, w_v, w_o):
    """Exact fp32 host computation for mask patterns the device kernel does
    not implement (never hit for the reference's causal mask)."""
    out = np.zeros((B, S, DM), np.float32)
    for b in range(B):
        heads = np.zeros((S, H * DH), np.float32)
        for h in range(H):
            q = embed[b] @ w_q[h]
            k = embed[b] @ w_k[h]
            v = embed[b] @ w_v[h]
            logits = (q @ k.T) * SCALE
            logits = np.where(mask[b], logits, -np.inf)
            logits -= logits.max(axis=-1, keepdims=True)
            p = np.exp(logits)
            p /= p.sum(axis=-1, keepdims=True)
            heads[:, h * DH : (h + 1) * DH] = p @ v
        out[b] = heads @ w_o
    return out


def _get_runner(causal):
    """Cached jitted sharded executor for the built Bass module.

    Mirrors bass2jax.run_bass_via_pjrt's multi-core path, but keeps the
    jitted callable so repeated kernel() calls skip re-tracing/compiling."""
    key = ("runner", causal)
    if key in _CACHE:
        return _CACHE[key]

    import jax
    from jax.experimental.shard_map import shard_map
    from jax.sharding import Mesh, PartitionSpec

    import concourse.mybir as mybir
    from concourse import bass2jax

    bass2jax.install_neuronx_cc_hook()
    nc = _get_nc(causal)
    partition_name = nc.partition_id_tensor.name if nc.partition_id_tensor else None
    in_names, out_names, out_avals, out_shapes = [], [], [], []
    for alloc in nc.m.functions[0].allocations:
        if not isinstance(alloc, mybir.MemoryLocationSet):
            continue
        name = alloc.memorylocations[0].name
        if alloc.kind == "ExternalInput":
            if name != partition_name:
                in_names.append(name)
        elif alloc.kind == "ExternalOutput":
            shape = tuple(alloc.tensor_shape)
            dtype = mybir.dt.np(alloc.dtype)
            out_names.append(name)
            out_avals.append(jax.core.ShapedArray(shape, dtype))
            out_shapes.append((shape, dtype))
    n_params = len(in_names)
    all_in_names = list(in_names) + list(out_names)
    if partition_name is not None:
        all_in_names.append(partition_name)

    def _body(*args):
        operands = list(args)
        if partition_name is not None:
            operands.append(bass2jax.partition_id_tensor())
        return tuple(
            bass2jax._bass_exec_p.bind(
                *operands,
                out_avals=tuple(out_avals),
                in_names=tuple(all_in_names),
                out_names=tuple(out_names),
                lowering_input_output_aliases=(),
                sim_require_finite=True,
                sim_require_nnan=True,
                nc=nc,
            )
        )

    devices = jax.devices()[:NCORES]
    mesh = Mesh(np.asarray(devices), ("core",))
    n_outs = len(out_names)
    sharded = jax.jit(
        shard_map(
            _body,
            mesh=mesh,
            in_specs=(PartitionSpec("core"),) * (n_params + n_outs),
            out_specs=(PartitionSpec("core"),) * n_outs,
            check_rep=False,
        ),
        keep_unused=True,
    )

    def run(in_maps):
        concat_in = [
            np.concatenate([np.asarray(in_maps[c][nm]) for c in range(NCORES)], axis=0)
            for nm in in_names
        ]
        concat_zeros = [
            np.zeros((NCORES * shape[0], *shape[1:]), dtype)
            for shape, dtype in out_shapes
        ]
        outs = sharded(*concat_in, *concat_zeros)
        return [
            {
                nm: np.asarray(outs[i]).reshape(NCORES, *out_shapes[i][0])[c]
                for i, nm in enumerate(out_names)
            }
            for c in range(NCORES)
        ]

    _CACHE[key] = run
    return run


def kernel(embed, mask, w_q, w_k, w_v, w_o):
    embed = np.asarray(embed, np.float32)
    mask = np.asarray(mask, bool)
    w_q = np.asarray(w_q, np.float32)
    w_k = np.asarray(w_k, np.float32)
    w_v = np.asarray(w_v, np.float32)
    w_o = np.asarray(w_o, np.float32)

    tril = np.tril(np.ones((S, S), dtype=bool))
    if all(np.array_equal(mask[b], tril) for b in range(B)):
        causal = True
    elif mask.all():
        causal = False
    else:
        return _numpy_fallback(embed, mask, w_q, w_k, w_v, w_o)

    run = _get_runner(causal)
    in_maps = _host_inputs(embed, w_q, w_k, w_v, w_o)
    results = run(in_maps)
    out = np.zeros((B, S, DM), np.float32)
    for b in range(B):
        out[b] = results[2 * b]["out"] + results[2 * b + 1]["out"]
    return out
